# revision 32
# baseline (speedup 1.0000x reference)
"""Trainium2 Bass kernel for a 3-layer FCL + size-5 sliding-window stack.

Reference computation (fp32):
    h = relu(x @ W1.T)          # [N, 10]
    t = relu(h @ W2.T + b2)     # [N, 5]
    out[n] = concat(t[n-2..n+2])  zero-padded  -> [N, 25]

Strategy (8 cores, data-parallel over rows, halo recomputed per core):
  - Host prep is layout/precision only: x is cast to fp8-e3m4 and
    pre-transposed so each core receives xT [320, 25088] (25000 own rows
    + 2-row halo each side, zero padded).  e3m4 quarters the HBM x read
    vs fp32 (8MB/core); e4m3 (which would enable DoubleRow 2x matmul)
    measures 2.3e-2 end-to-end - over the 2e-2 gate - so fp8 stays e3m4
    as the moving operand against bf16 stationary weights (bf16 speed,
    fp32 PSUM accumulation).
  - The PE instruction stream is organized so LDWEIGHTS never serializes
    with MATMUL (the baseline lost ~95ns per matmul to it): blocks are
    processed in GROUPS OF FOUR whose h accumulators live at partition
    strips {0-9, 32-41, 64-73, 96-105} of ONE psum bank, addressed with
    tile_position col strips.  Consecutive matmuls hit different 32-col
    sub-array quadrants, so each LDW loads into quadrants the in-flight
    matmul isn't using (the PE queue pulls LDW ahead - silicon feature).
  - L1's K=320 is split 128+128+64; the two 64-row tails of a block PAIR
    are packed into one full-height matmul with a block-diagonal
    [128, 64] weight (cols 0-9 <- rows 0-63, cols 32-41 <- rows 64-127),
    so L1 costs 2.5 passes/block instead of 3.  The c3 matmuls run FIRST
    with start=True: they write the full 2KB zero-region of their 64
    partitions, cleanly zeroing the unused strips (PSUM pending-zero
    semantics), so the later c1/c2 accumulates land on defined values.
  - L2 is ONE matmul per 4-block group: block-diagonal W2 [128, 100]
    (rows 32i+r -> cols 25i+m hold W2rep, the x5 window-replicated W2.T)
    against the group's relu'd h [128, 512] - 13 L2 matmuls instead of
    49.  DVE relu: one [128, 512] psum->sbuf bf16 op per group.  ACT
    bias+relu: one [100, 512] op per group (bias is per-partition).
  - The size-5 window gather costs nothing: with tT grouped as
    [100, 13*512] (partition 25i+5w+c = t[c, j], j = 2048g+512i+jj), the
    ENTIRE output store for window shift w is ONE strided DMA:
      src [[pitch,5(c)], [512,13(g)], [25*pitch,4(i)], [1,512(jj)]]
      dst [[OUTW,5], [2048,13], [512,4], [1,512]]  @ col 4-w
    writing out dram [25, 26628] with slack columns at both ends that
    absorb the halo/phantom-block spill (host reads cols [4, 25004)).
    5 DMAs per half (groups 0-6 issued mid-kernel, 7-12 at the end),
    spread over the sync/scalar/gpsimd rings: ~10 dma_start issues
    (~700ns each) instead of the baseline's ~19.
  - x loads: 2 DMAs per 4096-col superblock on the SP ring: one 3-dim
    AP for K-chunks 1-2 ([128, 8192] tile), one 4-dim AP that lands the
    64-row chunk-3 of block pairs on partition halves 0-63/64-127.
  - HAM warmup: 7 full-width matmuls on scratch lift the PE clock to
    2.4GHz during the initial DMA fill; after that the PE is gapless so
    the clock holds.  Total PE busy drops ~3x vs the baseline, which
    also keeps the chip's power manager from derating the clock
    mid-kernel (the baseline throttled to 1.2GHz for its last 35us).
  - Host unshard: concat per-core outT[:, 4:25004] along columns,
    upcast, transpose, patch the 4 global-edge window slots to exact
    zero (the reference zero-pads t, not x).
  - The ISA allows ONE sync-wait per instruction; a post-pass hoists any
    extra waits onto same-engine NoOps.
"""

import numpy as np
import ml_dtypes

import bass_rust
import concourse.bass as bass
import concourse.mybir as mybir
import concourse.tile as tile

# ---- problem constants (hardcoded per contract) ----
N = 200000
D = 320
D1 = 10
D2 = 5
W = 5
HALF = W // 2
NCORES = 8
ROWS = N // NCORES          # 25000 output rows per core
BLK = 512                   # t-cols per block (one PSUM bank)
NBLK = 49                   # 25088 padded t-cols per core
PAD = NBLK * BLK            # 25088
GRP = 4                     # blocks per group (4 psum strips)
NGRP = 13                   # 12 full groups + 1 single-block group
SB = 8                      # blocks per load superblock
NSB = 7                     # 6 full superblocks + 1 single-block
OUTW = 25092                # out dram cols: 4 head slack + 25088
NWARM = 7                   # warmup matmuls (~4.3us cold) lift HAM to 2.4GHz
F32 = mybir.dt.float32
BF16 = mybir.dt.bfloat16
FP8 = mybir.dt.float8e3
RELU = mybir.ActivationFunctionType.Relu
BF = ml_dtypes.bfloat16
F8 = ml_dtypes.float8_e3m4

# wc_sb column layout: [c1 0:10 | c2 10:20 | c3 pair-diag 20:84 | W2 blockdiag 84:184]
WC1, WC2, WC3, WL2, WCEND = 0, 10, 20, 84, 184

# load regions: groups [g0, g1), per-partition width (5120 per full
# group: [c1 2048 | c2 2048 | paired-c3 1024]; g12 is 1536 tight)
REGIONS = [(0, 4, 20480), (4, 8, 20480), (8, 10, 10240), (10, 12, 10240), (12, 13, 1536)]
XPTOT = sum(w for _, _, w in REGIONS)  # 63488 bytes per partition

_NC_CACHE = {}


def split_multiwaits(nc):
    """Walrus/ISA allows ONE sync-wait per instruction; Tile emits several.

    For every instruction with >1 wait, hoist all but the last wait onto
    fresh NoOps on the same engine immediately before it.  The engine
    stalls at the nops exactly as it would have at the instruction, so
    semantics are unchanged.
    """
    n_split = 0
    for bb in nc.main_func.blocks:
        insts = bb.instructions
        out = []
        changed = False
        for ins in insts:
            si = ins.sync_info
            waits = list(si.on_wait) if si is not None else []
            if len(waits) > 1:
                changed = True
                for w in waits[:-1]:
                    n_split += 1
                    nop = bass_rust.InstNoOp(name=f"wsplit-{n_split}")
                    nop.engine = ins.engine
                    nop.sync_info = bass_rust.SyncInfo(
                        on_wait=[w], on_update=[]
                    )
                    nc.inst_map[nop.name] = nop
                    out.append(nop)
                ins.sync_info = bass_rust.SyncInfo(
                    on_wait=[waits[-1]], on_update=list(si.on_update)
                )
            out.append(ins)
        if changed:
            bb.instructions = out
    return n_split


def build_nc():
    nc = bass.Bass("TRN2", target_bir_lowering=False, debug=False)

    # XP: per-superblock contiguous regions so every load DMA is one
    # fully-contiguous dram read: [chunk1 p-major 512KB | chunk2 512KB |
    # paired-chunk3 256KB] per superblock
    # need-ordered load regions (all on one queue; DMA throughput is
    # ~16 engines x packet/250ns, so 10-20KB per-partition runs reach
    # the ~390GB/s HBM wall on a single queue)
    xp_t = nc.dram_tensor("XP", [XPTOT * 128], FP8, kind="ExternalInput")
    wc_t = nc.dram_tensor("WC", [128, WCEND], BF16, kind="ExternalInput")
    b2r_t = nc.dram_tensor("B2R", [100], F32, kind="ExternalInput")
    # grouped output: outG[i, 5w+c, 512g+jj] = t[c, 2048g+512i+jj]
    # (host de-tiles the block grouping and applies the w-shift slices)
    outG_t = nc.dram_tensor(
        "outG", [GRP, W * D2, NGRP * BLK], BF16, kind="ExternalOutput"
    )

    with tile.TileContext(nc) as tc:
        with (
            tc.tile_pool(name="singles", bufs=1) as singles,
            tc.tile_pool(name="hspool", bufs=3) as hspool,
            tc.tile_pool(name="ps_h", bufs=3, space="PSUM") as ps_h,
            tc.tile_pool(name="ps_t", bufs=2, space="PSUM") as ps_t,
            tc.tile_pool(name="ps_w", bufs=1, space="PSUM") as ps_w,
        ):
            # ---- constants (one-time; scalar ring - the gpsimd DGE has
            # ~5us startup latency, so it only carries later superblocks) ----
            wc_sb = singles.tile([128, WCEND], BF16)
            nc.scalar.dma_start(out=wc_sb, in_=wc_t[:, :])
            # b2 pre-replicated by the host: b2r[25i+5w+c] = b2[c]
            # (a stride-0 broadcast DMA emits 100 4-byte packets that
            # clog the queue for ~10us - measured)
            b2r_sb = singles.tile([100, 1], F32)
            nc.scalar.dma_start(
                out=b2r_sb, in_=bass.AP(b2r_t, 0, [[1, 100], [1, 1]])
            )
            # persistent grouped t.T accumulator [100, 13, 512] bf16
            tT_g = singles.tile([100, NGRP, BLK], BF16)

            # ---- HAM warmup: full-width matmuls on scratch while the
            # first x loads stream in (PE is otherwise idle).  The HAM
            # window needs >=3.4us of sustained activity; 7 cold N=512
            # matmuls are ~4.3us. ----
            warm_sb = singles.tile([128, BLK], BF16)
            nc.vector.memset(warm_sb, 0.625)
            warm_ps = ps_w.tile([128, BLK], F32, tag="w")
            for i in range(NWARM):
                nc.tensor.matmul(
                    warm_ps, warm_sb[:, :128], warm_sb,
                    start=True, stop=True,
                )

            xa_sbs = {}     # superblock -> [128, 10240] fp8 (c1|c2|paired c3)
            h_pss = {}      # group -> h psum tile [128, 512]
            hs_sbs = {}     # group -> relu'd h [128, 512] bf16
            t_pss = {}      # group -> tT psum tile [100, 512]
            RINGS = [nc.sync, nc.scalar, nc.gpsimd]

            # ---- all x loads up front, need-ordered, on the SP ring ----
            xr_tiles = []
            off = 0
            for g0, g1, wdt in REGIONS:
                xr = singles.tile([128, wdt], FP8)
                nc.sync.dma_start(
                    out=xr,
                    in_=bass.AP(xp_t, off * 128, [[wdt, 128], [1, wdt]]),
                )
                xr_tiles.append(xr)
                off += wdt
            reg_of = {}
            for ri, (g0, g1, wdt) in enumerate(REGIONS):
                for g in range(g0, g1):
                    reg_of[g] = (ri, g - g0)

            def emit_group_mms(g):
                """10 matmuls for the 4 blocks of group g, strip-rotated."""
                nb = GRP if g < NGRP - 1 else 1
                ri, k = reg_of[g]
                xg = xr_tiles[ri]
                co1, co2, co3 = (
                    (5120 * k, 5120 * k + 2048, 5120 * k + 4096)
                    if nb == GRP else (0, BLK, 2 * BLK)
                )
                h_ps = ps_h.tile([128, BLK], F32, tag="h")
                # c3 pair matmuls first: start=True writes the strips'
                # full 2KB zero region (zeros where the diag weight is 0)
                for p in range(2 if nb == GRP else 1):
                    nc.tensor.matmul(
                        h_ps[64 * p : 64 * p + 64, :],
                        wc_sb[:, WC3:WL2],
                        xg[:, co3 + BLK * p : co3 + BLK * (p + 1)],
                        start=True, stop=False,
                        skip_group_check=True,
                        tile_position=(0, 64 * p),
                    )
                for co, w0 in ((co1, WC1), (co2, WC2)):
                    last = w0 == WC2
                    for i in range(nb):
                        nc.tensor.matmul(
                            h_ps[32 * i : 32 * i + D1, :],
                            wc_sb[:, w0 : w0 + D1],
                            xg[:, co + BLK * i : co + BLK * (i + 1)],
                            start=False, stop=last,
                            skip_group_check=True,
                            tile_position=(0, 32 * i),
                        )
                h_pss[g] = h_ps

            def emit_relu(g):
                """DVE: one relu+cast for the whole group's h strips."""
                nparts = 128 if g < NGRP - 1 else 42
                hs = hspool.tile([128, BLK], BF16, tag="hs")
                nc.vector.tensor_scalar_max(
                    hs[:nparts, :], h_pss[g][:nparts, :], 0.0
                )
                hs_sbs[g] = hs
                del h_pss[g]

            def emit_l2(g):
                """One stacked L2 matmul: block-diag W2 [128,100] @ h."""
                nk = 128 if g < NGRP - 1 else 42
                t_ps = ps_t.tile([100, BLK], F32, tag="t")
                nc.tensor.matmul(
                    t_ps, wc_sb[:nk, WL2:WCEND], hs_sbs[g][:nk, :],
                    start=True, stop=True,
                )
                t_pss[g] = t_ps
                del hs_sbs[g]

            def emit_act(g):
                """ACT: tT_g[:, g, :] = relu(t_ps + b2r)."""
                nc.scalar.activation(
                    tT_g[:, g, :],
                    t_pss[g],
                    RELU,
                    bias=b2r_sb,
                )
                del t_pss[g]

            def emit_store(g0, g1, r0):
                """Store tT_g groups [g0, g1) straight to dram in the
                grouped layout: one 2-dim DMA per strip i (big
                contiguous packets; the host de-tiles the grouping).
                Only scalar/gpsimd rings - sync is the load artery."""
                for i in range(GRP):
                    RINGS[1 + (r0 + i) % 2].dma_start(
                        out=outG_t[i, :, BLK * g0 : BLK * g1],
                        in_=tT_g[25 * i : 25 * i + 25, g0:g1, :],
                    )

            # ---- main loop (software-pipelined, one iteration per group) ----
            for g in range(NGRP):
                emit_group_mms(g)
                if g >= 1:
                    emit_l2(g - 1)
                emit_relu(g)
                if g >= 1:
                    emit_act(g - 1)
                if g == 12:
                    emit_store(0, 12, 1)
            emit_l2(NGRP - 1)
            emit_act(NGRP - 1)
            emit_store(NGRP - 1, NGRP, 1)

    split_multiwaits(nc)
    return nc


def make_shards(x):
    """Per-core xT [320, PAD] fp8-e3m4 shards, +-2 col halo, zero padded."""
    xbT = np.ascontiguousarray(x.astype(F8).T)  # [320, N]
    shards = []
    for c in range(NCORES):
        s = np.zeros((D, PAD), dtype=F8)
        lo = ROWS * c - HALF
        src_lo, src_hi = max(lo, 0), min(lo + PAD, N)
        s[:, src_lo - lo : src_lo - lo + (src_hi - src_lo)] = xbT[
            :, src_lo:src_hi
        ]
        shards.append(s)
    return shards


def make_xp(xbT):
    """Need-ordered flat load regions from one core's xT [320, PAD]."""
    out = []
    for g0, g1, wdt in REGIONS:
        reg = np.zeros((128, wdt), dtype=F8)
        for g in range(g0, g1):
            ncols = 2048 if g < NGRP - 1 else BLK
            cs = 2048 * g
            k = g - g0
            c0 = 5120 * k if g < NGRP - 1 else 0
            step = 2048 if g < NGRP - 1 else BLK
            reg[:, c0 : c0 + ncols] = xbT[0:128, cs : cs + ncols]
            reg[:, c0 + step : c0 + step + ncols] = xbT[128:256, cs : cs + ncols]
            c3 = xbT[256:320, cs : cs + ncols]
            if g < NGRP - 1:
                reg[:, c0 + 4096 : c0 + 5120] = (
                    c3.reshape(64, 2, 2, BLK).transpose(2, 0, 1, 3).reshape(128, 1024)
                )
            else:
                reg[0:64, c0 + 2 * BLK : c0 + 3 * BLK] = c3
        out.append(reg)
    return np.concatenate([r.reshape(-1) for r in out])


def make_wc(W1, W2):
    """Packed bf16 stationary weights [128, 184]."""
    wc = np.zeros((128, WCEND), dtype=np.float32)
    W1T = W1.T  # [320, 10]
    wc[:, WC1:WC1 + D1] = W1T[0:128]
    wc[:, WC2:WC2 + D1] = W1T[128:256]
    wc[0:64, WC3:WC3 + D1] = W1T[256:320]
    wc[64:128, WC3 + 32 : WC3 + 32 + D1] = W1T[256:320]
    W2rep = np.tile(W2.T, (1, W))  # [10, 25]
    for i in range(4):
        wc[32 * i : 32 * i + D1, WL2 + 25 * i : WL2 + 25 * (i + 1)] = W2rep
    return np.ascontiguousarray(wc.astype(BF))


def _patch_edges(out):
    # the reference zero-pads t, not x: window slots that fall outside
    # [0, N) must be exactly zero.
    out[0, : 2 * D2] = 0.0
    out[1, :D2] = 0.0
    out[N - 2, 4 * D2 :] = 0.0
    out[N - 1, 3 * D2 :] = 0.0
    return out


def run(inputs, trace=False):
    from concourse.bass_utils import run_bass_kernel_spmd

    x = np.ascontiguousarray(np.asarray(inputs["x"], dtype=np.float32))
    W1 = np.asarray(inputs["W1"], dtype=np.float32)
    W2 = np.asarray(inputs["W2"], dtype=np.float32)
    b2 = np.ascontiguousarray(np.asarray(inputs["b2"], dtype=np.float32))
    assert x.shape == (N, D)

    WC = make_wc(W1, W2)

    if "nc" not in _NC_CACHE:
        _NC_CACHE["nc"] = build_nc()
    nc = _NC_CACHE["nc"]

    B2R = np.ascontiguousarray(np.tile(b2, 20))
    in_maps = [
        {"XP": make_xp(s), "WC": WC, "B2R": B2R} for s in make_shards(x)
    ]
    res = run_bass_kernel_spmd(nc, in_maps, list(range(NCORES)), trace=trace)
    cores = []
    for c in range(NCORES):
        og = np.asarray(res.results[c]["outG"])  # [4, 25, 13*512] bf16
        # de-tile the block grouping: [i, r, 512g+jj] -> [r, 2048g+512i+jj]
        flat = np.ascontiguousarray(
            og.reshape(GRP, 25, NGRP, BLK).transpose(1, 2, 0, 3)
        ).reshape(25, GRP * NGRP * BLK)
        core = np.empty((25, ROWS), dtype=og.dtype)
        for w in range(W):  # out[5w+c, n] = t[c, n+w] = flat[5w+c, n+w]
            core[5 * w : 5 * w + D2] = flat[5 * w : 5 * w + D2, w : w + ROWS]
        cores.append(core)
    out = np.ascontiguousarray(
        np.concatenate(cores, axis=1).astype(np.float32).T
    )
    return _patch_edges(out), res


def kernel(**inputs):
    out, _ = run(inputs, trace=False)
    return out


# revision 33
# speedup vs baseline: 1.5203x; 1.5203x over previous
"""Trainium2 Bass kernel for a 3-layer FCL + size-5 sliding-window stack.

Reference computation (fp32):
    h = relu(x @ W1.T)          # [N, 10]
    t = relu(h @ W2.T + b2)     # [N, 5]
    out[n] = concat(t[n-2..n+2])  zero-padded  -> [N, 25]

Strategy (8 cores, data-parallel over rows, halo recomputed per core):
  - Host prep is layout/precision only: x is cast to fp8-e3m4 and
    pre-transposed so each core receives xT [320, 25088] (25000 own rows
    + 2-row halo each side, zero padded).  e3m4 quarters the HBM x read
    vs fp32 (8MB/core); e4m3 (which would enable DoubleRow 2x matmul)
    measures 2.3e-2 end-to-end - over the 2e-2 gate - so fp8 stays e3m4
    as the moving operand against bf16 stationary weights (bf16 speed,
    fp32 PSUM accumulation).
  - The PE instruction stream is organized so LDWEIGHTS never serializes
    with MATMUL (the baseline lost ~95ns per matmul to it): blocks are
    processed in GROUPS OF FOUR whose h accumulators live at partition
    strips {0-9, 32-41, 64-73, 96-105} of ONE psum bank, addressed with
    tile_position col strips.  Consecutive matmuls hit different 32-col
    sub-array quadrants, so each LDW loads into quadrants the in-flight
    matmul isn't using (the PE queue pulls LDW ahead - silicon feature).
  - L1's K=320 is split 128+128+64; the two 64-row tails of a block PAIR
    are packed into one full-height matmul with a block-diagonal
    [128, 64] weight (cols 0-9 <- rows 0-63, cols 32-41 <- rows 64-127),
    so L1 costs 2.5 passes/block instead of 3.  The c3 matmuls run FIRST
    with start=True: they write the full 2KB zero-region of their 64
    partitions, cleanly zeroing the unused strips (PSUM pending-zero
    semantics), so the later c1/c2 accumulates land on defined values.
  - L2 is ONE matmul per 4-block group: block-diagonal W2 [128, 100]
    (rows 32i+r -> cols 25i+m hold W2rep, the x5 window-replicated W2.T)
    against the group's relu'd h [128, 512] - 13 L2 matmuls instead of
    49.  DVE relu: one [128, 512] psum->sbuf bf16 op per group.  ACT
    bias+relu: one [100, 512] op per group (bias is per-partition).
  - The size-5 window gather costs nothing: with tT grouped as
    [100, 13*512] (partition 25i+5w+c = t[c, j], j = 2048g+512i+jj), the
    ENTIRE output store for window shift w is ONE strided DMA:
      src [[pitch,5(c)], [512,13(g)], [25*pitch,4(i)], [1,512(jj)]]
      dst [[OUTW,5], [2048,13], [512,4], [1,512]]  @ col 4-w
    writing out dram [25, 26628] with slack columns at both ends that
    absorb the halo/phantom-block spill (host reads cols [4, 25004)).
    5 DMAs per half (groups 0-6 issued mid-kernel, 7-12 at the end),
    spread over the sync/scalar/gpsimd rings: ~10 dma_start issues
    (~700ns each) instead of the baseline's ~19.
  - x loads: 2 DMAs per 4096-col superblock on the SP ring: one 3-dim
    AP for K-chunks 1-2 ([128, 8192] tile), one 4-dim AP that lands the
    64-row chunk-3 of block pairs on partition halves 0-63/64-127.
  - HAM warmup: 7 full-width matmuls on scratch lift the PE clock to
    2.4GHz during the initial DMA fill; after that the PE is gapless so
    the clock holds.  Total PE busy drops ~3x vs the baseline, which
    also keeps the chip's power manager from derating the clock
    mid-kernel (the baseline throttled to 1.2GHz for its last 35us).
  - Host unshard: concat per-core outT[:, 4:25004] along columns,
    upcast, transpose, patch the 4 global-edge window slots to exact
    zero (the reference zero-pads t, not x).
  - The ISA allows ONE sync-wait per instruction; a post-pass hoists any
    extra waits onto same-engine NoOps.
"""

import numpy as np
import ml_dtypes

import bass_rust
import concourse.bass as bass
import concourse.mybir as mybir
import concourse.tile as tile

# ---- problem constants (hardcoded per contract) ----
N = 200000
D = 320
D1 = 10
D2 = 5
W = 5
HALF = W // 2
NCORES = 8
ROWS = N // NCORES          # 25000 output rows per core
BLK = 512                   # t-cols per block (one PSUM bank)
NBLK = 49                   # 25088 padded t-cols per core
PAD = NBLK * BLK            # 25088
GRP = 4                     # blocks per group (4 psum strips)
NGRP = 13                   # 12 full groups + 1 single-block group
SB = 8                      # blocks per load superblock
NSB = 7                     # 6 full superblocks + 1 single-block
OUTW = 25092                # out dram cols: 4 head slack + 25088
NWARM = 7                   # warmup matmuls (~4.3us cold) lift HAM to 2.4GHz
F32 = mybir.dt.float32
BF16 = mybir.dt.bfloat16
FP8 = mybir.dt.float8e3
RELU = mybir.ActivationFunctionType.Relu
BF = ml_dtypes.bfloat16
F8 = ml_dtypes.float8_e3m4

# wc_sb column layout: [c1 0:10 | c2 10:20 | c3 pair-diag 20:84 | W2 blockdiag 84:184]
WC1, WC2, WC3, WL2, WCEND = 0, 10, 20, 84, 184

# load regions: groups [g0, g1), per-partition width (5120 per full
# group: [c1 2048 | c2 2048 | paired-c3 1024]; g12 is 1536 tight)
REGIONS = [(0, 4, 20480), (4, 8, 20480), (8, 10, 10240), (10, 12, 10240), (12, 13, 1536)]
XPTOT = sum(w for _, _, w in REGIONS)  # 63488 bytes per partition

_NC_CACHE = {}


def split_multiwaits(nc):
    """Walrus/ISA allows ONE sync-wait per instruction; Tile emits several.

    For every instruction with >1 wait, hoist all but the last wait onto
    fresh NoOps on the same engine immediately before it.  The engine
    stalls at the nops exactly as it would have at the instruction, so
    semantics are unchanged.
    """
    n_split = 0
    for bb in nc.main_func.blocks:
        insts = bb.instructions
        out = []
        changed = False
        for ins in insts:
            si = ins.sync_info
            waits = list(si.on_wait) if si is not None else []
            if len(waits) > 1:
                changed = True
                for w in waits[:-1]:
                    n_split += 1
                    nop = bass_rust.InstNoOp(name=f"wsplit-{n_split}")
                    nop.engine = ins.engine
                    nop.sync_info = bass_rust.SyncInfo(
                        on_wait=[w], on_update=[]
                    )
                    nc.inst_map[nop.name] = nop
                    out.append(nop)
                ins.sync_info = bass_rust.SyncInfo(
                    on_wait=[waits[-1]], on_update=list(si.on_update)
                )
            out.append(ins)
        if changed:
            bb.instructions = out
    return n_split


def build_nc():
    nc = bass.Bass("TRN2", target_bir_lowering=False, debug=False)

    # XP: per-superblock contiguous regions so every load DMA is one
    # fully-contiguous dram read: [chunk1 p-major 512KB | chunk2 512KB |
    # paired-chunk3 256KB] per superblock
    # need-ordered load regions (all on one queue; DMA throughput is
    # ~16 engines x packet/250ns, so 10-20KB per-partition runs reach
    # the ~390GB/s HBM wall on a single queue)
    xp_t = nc.dram_tensor("XP", [XPTOT * 128], FP8, kind="ExternalInput")
    wc_t = nc.dram_tensor("WC", [128, WCEND], BF16, kind="ExternalInput")
    b2r_t = nc.dram_tensor("B2R", [100], F32, kind="ExternalInput")
    # grouped output: outG[i, 5w+c, 512g+jj] = t[c, 2048g+512i+jj]
    # (host de-tiles the block grouping and applies the w-shift slices)
    outG_t = nc.dram_tensor(
        "outG", [GRP, W * D2, NGRP * BLK], BF16, kind="ExternalOutput"
    )

    with tile.TileContext(nc) as tc:
        with (
            tc.tile_pool(name="singles", bufs=1) as singles,
            tc.tile_pool(name="xr0", bufs=1) as xr0p,
            tc.tile_pool(name="xr1", bufs=1) as xr1p,
            tc.tile_pool(name="xr2", bufs=1) as xr2p,
            tc.tile_pool(name="xr3", bufs=1) as xr3p,
            tc.tile_pool(name="xr4", bufs=1) as xr4p,
            tc.tile_pool(name="hspool", bufs=3) as hspool,
            tc.tile_pool(name="ps_h", bufs=3, space="PSUM") as ps_h,
            tc.tile_pool(name="ps_t", bufs=2, space="PSUM") as ps_t,
            tc.tile_pool(name="ps_w", bufs=1, space="PSUM") as ps_w,
        ):
            # ---- constants (one-time; scalar ring - the gpsimd DGE has
            # ~5us startup latency, so it only carries later superblocks) ----
            wc_sb = singles.tile([128, WCEND], BF16)
            nc.scalar.dma_start(out=wc_sb, in_=wc_t[:, :])
            # b2 pre-replicated by the host: b2r[25i+5w+c] = b2[c]
            # (a stride-0 broadcast DMA emits 100 4-byte packets that
            # clog the queue for ~10us - measured)
            b2r_sb = singles.tile([100, 1], F32)
            nc.scalar.dma_start(
                out=b2r_sb, in_=bass.AP(b2r_t, 0, [[1, 100], [1, 1]])
            )
            # persistent grouped t.T accumulator [100, 13, 512] bf16
            tT_g = singles.tile([100, NGRP, BLK], BF16)

            # ---- HAM warmup: full-width matmuls on scratch while the
            # first x loads stream in (PE is otherwise idle).  The HAM
            # window needs >=3.4us of sustained activity; 7 cold N=512
            # matmuls are ~4.3us. ----
            warm_sb = singles.tile([128, BLK], BF16)
            nc.vector.memset(warm_sb, 0.625)
            warm_ps = ps_w.tile([128, BLK], F32, tag="w")
            for i in range(NWARM):
                nc.tensor.matmul(
                    warm_ps, warm_sb[:, :128], warm_sb,
                    start=True, stop=True,
                )

            xa_sbs = {}     # superblock -> [128, 10240] fp8 (c1|c2|paired c3)
            h_pss = {}      # group -> h psum tile [128, 512]
            hs_sbs = {}     # group -> relu'd h [128, 512] bf16
            t_pss = {}      # group -> tT psum tile [100, 512]
            RINGS = [nc.sync, nc.scalar, nc.gpsimd]

            # ---- all x loads up front, need-ordered, on the SP ring ----
            xr_tiles = []
            off = 0
            xr_pools = [xr0p, xr1p, xr2p, xr3p, xr4p]
            for (g0, g1, wdt), xrp in zip(REGIONS, xr_pools):
                xr = xrp.tile([128, wdt], FP8)
                nc.sync.dma_start(
                    out=xr,
                    in_=bass.AP(xp_t, off * 128, [[wdt, 128], [1, wdt]]),
                )
                xr_tiles.append(xr)
                off += wdt
            reg_of = {}
            for ri, (g0, g1, wdt) in enumerate(REGIONS):
                for g in range(g0, g1):
                    reg_of[g] = (ri, g - g0)

            def emit_group_mms(g):
                """10 matmuls for the 4 blocks of group g, strip-rotated."""
                nb = GRP if g < NGRP - 1 else 1
                ri, k = reg_of[g]
                xg = xr_tiles[ri]
                co1, co2, co3 = (
                    (5120 * k, 5120 * k + 2048, 5120 * k + 4096)
                    if nb == GRP else (0, BLK, 2 * BLK)
                )
                h_ps = ps_h.tile([128, BLK], F32, tag="h")
                # c3 pair matmuls first: start=True writes the strips'
                # full 2KB zero region (zeros where the diag weight is 0)
                for p in range(2 if nb == GRP else 1):
                    nc.tensor.matmul(
                        h_ps[64 * p : 64 * p + 64, :],
                        wc_sb[:, WC3:WL2],
                        xg[:, co3 + BLK * p : co3 + BLK * (p + 1)],
                        start=True, stop=False,
                        skip_group_check=True,
                        tile_position=(0, 64 * p),
                    )
                for co, w0 in ((co1, WC1), (co2, WC2)):
                    last = w0 == WC2
                    for i in range(nb):
                        nc.tensor.matmul(
                            h_ps[32 * i : 32 * i + D1, :],
                            wc_sb[:, w0 : w0 + D1],
                            xg[:, co + BLK * i : co + BLK * (i + 1)],
                            start=False, stop=last,
                            skip_group_check=True,
                            tile_position=(0, 32 * i),
                        )
                h_pss[g] = h_ps

            def emit_relu(g):
                """DVE: one relu+cast for the whole group's h strips."""
                nparts = 128 if g < NGRP - 1 else 42
                hs = hspool.tile([128, BLK], BF16, tag="hs")
                nc.vector.tensor_scalar_max(
                    hs[:nparts, :], h_pss[g][:nparts, :], 0.0
                )
                hs_sbs[g] = hs
                del h_pss[g]

            def emit_l2(g):
                """One stacked L2 matmul: block-diag W2 [128,100] @ h."""
                nk = 128 if g < NGRP - 1 else 42
                t_ps = ps_t.tile([100, BLK], F32, tag="t")
                nc.tensor.matmul(
                    t_ps, wc_sb[:nk, WL2:WCEND], hs_sbs[g][:nk, :],
                    start=True, stop=True,
                )
                t_pss[g] = t_ps
                del hs_sbs[g]

            def emit_act(g):
                """ACT: tT_g[:, g, :] = relu(t_ps + b2r)."""
                nc.scalar.activation(
                    tT_g[:, g, :],
                    t_pss[g],
                    RELU,
                    bias=b2r_sb,
                )
                del t_pss[g]

            def emit_store(g0, g1, r0):
                """Store tT_g groups [g0, g1) straight to dram in the
                grouped layout: one 2-dim DMA per strip i (big
                contiguous packets; the host de-tiles the grouping).
                Only scalar/gpsimd rings - sync is the load artery."""
                for i in range(GRP):
                    RINGS[1 + (r0 + i) % 2].dma_start(
                        out=outG_t[i, :, BLK * g0 : BLK * g1],
                        in_=tT_g[25 * i : 25 * i + 25, g0:g1, :],
                    )

            # ---- main loop (software-pipelined, one iteration per group) ----
            for g in range(NGRP):
                emit_group_mms(g)
                if g >= 1:
                    emit_l2(g - 1)
                emit_relu(g)
                if g >= 1:
                    emit_act(g - 1)
                if g == 12:
                    emit_store(0, 12, 1)
            emit_l2(NGRP - 1)
            emit_act(NGRP - 1)
            emit_store(NGRP - 1, NGRP, 1)

    split_multiwaits(nc)
    return nc


def make_shards(x):
    """Per-core xT [320, PAD] fp8-e3m4 shards, +-2 col halo, zero padded."""
    xbT = np.ascontiguousarray(x.astype(F8).T)  # [320, N]
    shards = []
    for c in range(NCORES):
        s = np.zeros((D, PAD), dtype=F8)
        lo = ROWS * c - HALF
        src_lo, src_hi = max(lo, 0), min(lo + PAD, N)
        s[:, src_lo - lo : src_lo - lo + (src_hi - src_lo)] = xbT[
            :, src_lo:src_hi
        ]
        shards.append(s)
    return shards


def make_xp(xbT):
    """Need-ordered flat load regions from one core's xT [320, PAD]."""
    out = []
    for g0, g1, wdt in REGIONS:
        reg = np.zeros((128, wdt), dtype=F8)
        for g in range(g0, g1):
            ncols = 2048 if g < NGRP - 1 else BLK
            cs = 2048 * g
            k = g - g0
            c0 = 5120 * k if g < NGRP - 1 else 0
            step = 2048 if g < NGRP - 1 else BLK
            reg[:, c0 : c0 + ncols] = xbT[0:128, cs : cs + ncols]
            reg[:, c0 + step : c0 + step + ncols] = xbT[128:256, cs : cs + ncols]
            c3 = xbT[256:320, cs : cs + ncols]
            if g < NGRP - 1:
                reg[:, c0 + 4096 : c0 + 5120] = (
                    c3.reshape(64, 2, 2, BLK).transpose(2, 0, 1, 3).reshape(128, 1024)
                )
            else:
                reg[0:64, c0 + 2 * BLK : c0 + 3 * BLK] = c3
        out.append(reg)
    return np.concatenate([r.reshape(-1) for r in out])


def make_wc(W1, W2):
    """Packed bf16 stationary weights [128, 184]."""
    wc = np.zeros((128, WCEND), dtype=np.float32)
    W1T = W1.T  # [320, 10]
    wc[:, WC1:WC1 + D1] = W1T[0:128]
    wc[:, WC2:WC2 + D1] = W1T[128:256]
    wc[0:64, WC3:WC3 + D1] = W1T[256:320]
    wc[64:128, WC3 + 32 : WC3 + 32 + D1] = W1T[256:320]
    W2rep = np.tile(W2.T, (1, W))  # [10, 25]
    for i in range(4):
        wc[32 * i : 32 * i + D1, WL2 + 25 * i : WL2 + 25 * (i + 1)] = W2rep
    return np.ascontiguousarray(wc.astype(BF))


def _patch_edges(out):
    # the reference zero-pads t, not x: window slots that fall outside
    # [0, N) must be exactly zero.
    out[0, : 2 * D2] = 0.0
    out[1, :D2] = 0.0
    out[N - 2, 4 * D2 :] = 0.0
    out[N - 1, 3 * D2 :] = 0.0
    return out


def run(inputs, trace=False):
    from concourse.bass_utils import run_bass_kernel_spmd

    x = np.ascontiguousarray(np.asarray(inputs["x"], dtype=np.float32))
    W1 = np.asarray(inputs["W1"], dtype=np.float32)
    W2 = np.asarray(inputs["W2"], dtype=np.float32)
    b2 = np.ascontiguousarray(np.asarray(inputs["b2"], dtype=np.float32))
    assert x.shape == (N, D)

    WC = make_wc(W1, W2)

    if "nc" not in _NC_CACHE:
        _NC_CACHE["nc"] = build_nc()
    nc = _NC_CACHE["nc"]

    B2R = np.ascontiguousarray(np.tile(b2, 20))
    in_maps = [
        {"XP": make_xp(s), "WC": WC, "B2R": B2R} for s in make_shards(x)
    ]
    res = run_bass_kernel_spmd(nc, in_maps, list(range(NCORES)), trace=trace)
    cores = []
    for c in range(NCORES):
        og = np.asarray(res.results[c]["outG"])  # [4, 25, 13*512] bf16
        # de-tile the block grouping: [i, r, 512g+jj] -> [r, 2048g+512i+jj]
        flat = np.ascontiguousarray(
            og.reshape(GRP, 25, NGRP, BLK).transpose(1, 2, 0, 3)
        ).reshape(25, GRP * NGRP * BLK)
        core = np.empty((25, ROWS), dtype=og.dtype)
        for w in range(W):  # out[5w+c, n] = t[c, n+w] = flat[5w+c, n+w]
            core[5 * w : 5 * w + D2] = flat[5 * w : 5 * w + D2, w : w + ROWS]
        cores.append(core)
    out = np.ascontiguousarray(
        np.concatenate(cores, axis=1).astype(np.float32).T
    )
    return _patch_edges(out), res


def kernel(**inputs):
    out, _ = run(inputs, trace=False)
    return out
